# revision 5
# baseline (speedup 1.0000x reference)
"""Correlation-cycle (Chamfer) loss kernel for Trainium2, 8 NeuronCores.

reference:  P[b,i,j] = ||x_i||^2 + ||y_j||^2 - 2 x_i.y_j   (x=corr_pred, y=corr_target)
            out = (mean_{b,j} min_i clip(P,0,100) + mean_{b,i} min_j clip(P,0,100)) / B

Sharding: B=4 batches x 2 i-halves -> 8 cores. Each core owns an x-half
(2048 rows) and the full y (4096 rows) of one batch.

Scheme "v2" (default): ONE matmul orientation [i x j]; each PSUM group
[128 x gw] (= -2*z, bf16 inputs / fp32 accumulate) is consumed by exactly
two DVE ops:
  tensor_tensor_reduce: u_bf16 = psum + y2bc ( = y2_j - 2 z_ij );
                        rowacc[:, k] = min_j u            (row mins, fused)
  scalar_tensor_tensor: colacc = min(colacc, u + x2_i)    (col mins; first
                        i-chunk uses tensor_scalar copy -> no memset)
All inputs packed into one bf16 dram tensor (chunked DMA for queue
parallelism), outputs packed into one bf16 tensor. Host: min over
lanes/cores (+x2_i for rows), clip(0,100) -- clip commutes with min --
then means.  Numerics: nearly all mins clip to 100 here, so bf16
everywhere is far inside the 2e-2 tolerance.

Scheme "hybrid": previous baseline (3 DVE ops/group, separate DMAs).
"""

import numpy as np
import ml_dtypes

import concourse.bass as bass
import concourse.mybir as mybir
import concourse.tile as tile
from concourse import bacc
from concourse.bass_utils import run_bass_kernel_spmd

BF16 = ml_dtypes.bfloat16
F32 = np.float32

B, N, D = 4, 4096, 128
NCORES = 8
NI = N // 2          # per-core i range (half a batch)
NJ = N               # full j range
GW = 2048            # psum group width (4 banks)
MMW = 512            # matmul moving width (1 bank)
BIG = 1.0e38         # accumulator init (min identity; fits bf16)

AluOp = mybir.AluOpType
ActFn = mybir.ActivationFunctionType


def build(ni=NI, nj=NJ, gw=GW, reps=1, scheme="v2"):
    if scheme == "hybrid":
        return build_hybrid(ni, nj, min(gw, 2048), reps)
    n_ic = ni // 128
    n_jg = nj // gw
    nq = gw // MMW
    in_w = ni + 2 * nj                  # xT | m2yT | y2bc
    out_w = nj + n_ic * n_jg            # colB | rowB

    nc = bacc.Bacc("TRN2", target_bir_lowering=False, debug=False,
                   enable_asserts=False, num_devices=NCORES)
    f32 = mybir.dt.float32
    bf16 = mybir.dt.bfloat16

    in_d = nc.dram_tensor("inp", [128, in_w], bf16, kind="ExternalInput")
    x2c_d = nc.dram_tensor("x2c", [128, n_ic], f32, kind="ExternalInput")
    out_d = nc.dram_tensor("outp", [128, out_w], bf16, kind="ExternalOutput")

    psum_bufs = 2 if gw <= 2048 else 1

    with tile.TileContext(nc) as tc:
        with (
            tc.tile_pool(name="persist", bufs=1) as persist,
            tc.tile_pool(name="psum", bufs=psum_bufs, space="PSUM") as psum_pool,
            tc.tile_pool(name="u", bufs=2) as upool,
        ):
            inp = persist.tile([128, in_w], bf16, name="inp")
            x2c = persist.tile([128, n_ic], f32, name="x2c")
            outp = persist.tile([128, out_w], bf16, name="outp")
            xT = inp[:, 0:ni]
            m2yT = inp[:, ni:ni + nj]
            y2bc = inp[:, ni + nj:ni + 2 * nj]

            ck = 2048
            for c0 in range(0, in_w, ck):
                ce = min(c0 + ck, in_w)
                nc.sync.dma_start(out=inp[:, c0:ce], in_=in_d[:, c0:ce])
            nc.sync.dma_start(out=x2c[:, :], in_=x2c_d[:, :])

            def emit_body():
                for ic in range(n_ic):
                    for jg in range(n_jg):
                        sl = slice(jg * gw, (jg + 1) * gw)
                        psum = psum_pool.tile([128, gw], f32, tag="ps", name="ps")
                        for q in range(nq):
                            j0 = jg * gw + q * MMW
                            nc.tensor.matmul(
                                psum[:, q * MMW:(q + 1) * MMW],
                                xT[:, ic * 128:(ic + 1) * 128],
                                m2yT[:, j0:j0 + MMW])
                        u = upool.tile([128, gw], bf16, tag="u", name="u")
                        k = nj + ic * n_jg + jg
                        nc.vector.tensor_tensor_reduce(
                            u[:, :], psum[:, :], y2bc[:, sl], 1.0, BIG,
                            AluOp.add, AluOp.min, outp[:, k:k + 1])
                        if ic == 0:
                            nc.vector.tensor_scalar(
                                outp[:, sl], u[:, :], x2c[:, 0:1], None,
                                AluOp.add)
                        else:
                            nc.vector.scalar_tensor_tensor(
                                outp[:, sl], u[:, :], x2c[:, ic:ic + 1],
                                outp[:, sl], AluOp.add, AluOp.min)

            if reps > 1:
                with tc.For_i(0, reps, 1,
                              hint_engines=(mybir.EngineType.PE,
                                            mybir.EngineType.DVE)):
                    emit_body()
            else:
                emit_body()

            for c0 in range(0, out_w, ck):
                ce = min(c0 + ck, out_w)
                nc.sync.dma_start(out=out_d[:, c0:ce], in_=outp[:, c0:ce])

    nc.compile()
    return nc


def build_hybrid(ni, nj, gw, reps):
    """Previous baseline: 3 DVE ops/group, separate dram tensors."""
    n_ic = ni // 128
    n_jg = nj // gw

    nc = bacc.Bacc("TRN2", target_bir_lowering=False, debug=False,
                   enable_asserts=False, num_devices=NCORES)
    f32 = mybir.dt.float32
    bf16 = mybir.dt.bfloat16

    xT_d = nc.dram_tensor("xT", [128, ni], bf16, kind="ExternalInput")
    m2yT_d = nc.dram_tensor("m2yT", [128, nj], bf16, kind="ExternalInput")
    x2c_d = nc.dram_tensor("x2c", [128, n_ic], f32, kind="ExternalInput")
    y2bc_d = nc.dram_tensor("y2bc", [128, nj], bf16, kind="ExternalInput")
    colB_d = nc.dram_tensor("colB", [128, nj], bf16, kind="ExternalOutput")
    rowR_d = nc.dram_tensor("rowR", [128, n_ic * n_jg], f32, kind="ExternalOutput")

    with tile.TileContext(nc) as tc:
        with (
            tc.tile_pool(name="persist", bufs=1) as persist,
            tc.tile_pool(name="psum", bufs=2, space="PSUM") as psum_pool,
            tc.tile_pool(name="u", bufs=3) as upool,
        ):
            xT = persist.tile([128, ni], bf16, name="xT")
            m2yT = persist.tile([128, nj], bf16, name="m2yT")
            x2c = persist.tile([128, n_ic], f32, name="x2c")
            y2bc = persist.tile([128, nj], bf16, name="y2bc")
            colB = persist.tile([128, nj], bf16, name="colB")
            rowR = persist.tile([128, n_ic * n_jg], f32, name="rowR")

            nc.sync.dma_start(out=xT[:, :], in_=xT_d[:, :])
            ck = min(2048, nj)
            for c0 in range(0, nj, ck):
                nc.sync.dma_start(out=m2yT[:, c0:c0 + ck], in_=m2yT_d[:, c0:c0 + ck])
                nc.sync.dma_start(out=y2bc[:, c0:c0 + ck], in_=y2bc_d[:, c0:c0 + ck])
            nc.sync.dma_start(out=x2c[:, :], in_=x2c_d[:, :])
            nc.vector.memset(colB[:, :], BIG)

            def emit_body():
                for ic in range(n_ic):
                    for jg in range(n_jg):
                        sl = slice(jg * gw, (jg + 1) * gw)
                        psum = psum_pool.tile([128, gw], f32, tag="ps", name="ps")
                        for q in range(gw // MMW):
                            j0 = jg * gw + q * MMW
                            nc.tensor.matmul(
                                psum[:, q * MMW:(q + 1) * MMW],
                                xT[:, ic * 128:(ic + 1) * 128],
                                m2yT[:, j0:j0 + MMW])
                        u = upool.tile([128, gw], bf16, tag="u", name="u")
                        nc.vector.tensor_tensor(
                            u[:, :], psum[:, :], y2bc[:, sl], AluOp.add)
                        k = ic * n_jg + jg
                        nc.vector.tensor_reduce(
                            rowR[:, k:k + 1], u[:, :],
                            mybir.AxisListType.X, AluOp.min)
                        nc.vector.scalar_tensor_tensor(
                            colB[:, sl], u[:, :], x2c[:, ic:ic + 1],
                            colB[:, sl], AluOp.add, AluOp.min)

            if reps > 1:
                with tc.For_i(0, reps, 1,
                              hint_engines=(mybir.EngineType.PE,
                                            mybir.EngineType.DVE)):
                    emit_body()
            else:
                emit_body()

            for c0 in range(0, nj, ck):
                nc.sync.dma_start(out=colB_d[:, c0:c0 + ck], in_=colB[:, c0:c0 + ck])
            nc.sync.dma_start(out=rowR_d[:, :], in_=rowR[:, :])

    nc.compile()
    return nc


def host_prep(x, y, scheme="v2", gw=GW):
    """Per-core input maps. Core c: batch c//2, i-half c%2."""
    x = np.ascontiguousarray(np.asarray(x, F32))
    y = np.ascontiguousarray(np.asarray(y, F32))
    x16 = x.astype(BF16)
    y16 = y.astype(BF16)
    m2y16 = (y16.astype(F32) * -2.0).astype(BF16)          # exact in bf16
    x2 = (x16.astype(F32) ** 2).sum(-1)                    # [B, N]
    y2 = (y16.astype(F32) ** 2).sum(-1)
    n_ic = NI // 128
    in_maps = []
    for c in range(NCORES):
        b, h = divmod(c, 2)
        i0 = h * NI
        if scheme == "v2":
            xTc = x16[b, i0:i0 + NI, :].T                          # [128, NI]
            y2bc = np.broadcast_to(y2[b].astype(BF16), (128, N))   # [128, N]
            inp = np.concatenate([xTc, m2y16[b].T, y2bc], axis=1)
            m = {"inp": np.ascontiguousarray(inp),
                 "x2c": np.ascontiguousarray(
                     x2[b, i0:i0 + NI].reshape(n_ic, 128).T)}
        else:
            m = {
                "xT": np.ascontiguousarray(x16[b, i0:i0 + NI, :].T),
                "m2yT": np.ascontiguousarray(m2y16[b].T),
                "x2c": np.ascontiguousarray(
                    x2[b, i0:i0 + NI].reshape(n_ic, 128).T),
                "y2bc": np.ascontiguousarray(
                    np.broadcast_to(y2[b].astype(BF16), (128, N))),
            }
        in_maps.append(m)
    return in_maps, x2, y2


def combine(results, x2, y2, scheme="v2", gw=GW):
    n_ic = NI // 128
    n_jg = N // gw
    col_mins = np.empty((B, N), F32)
    row_mins = np.empty((B, N), F32)
    for b in range(B):
        cores = [results[2 * b], results[2 * b + 1]]
        if scheme == "v2":
            col = np.minimum.reduce(
                [r["outp"][:, :N].astype(F32).min(0) for r in cores])
            col_mins[b] = np.clip(col, 0.0, 100.0)
            for h, r in enumerate(cores):
                rr = r["outp"][:, N:].astype(F32)      # [128, n_ic*n_jg]
                rr = rr.reshape(128, n_ic, n_jg).min(axis=2)
                row = rr.T.reshape(-1)                 # [NI], i = ic*128 + lane
                i0 = h * NI
                row_mins[b, i0:i0 + NI] = np.clip(
                    row + x2[b, i0:i0 + NI], 0.0, 100.0)
        else:
            col = np.minimum.reduce([r["colB"].astype(F32).min(0) for r in cores])
            col_mins[b] = np.clip(col, 0.0, 100.0)
            for h, r in enumerate(cores):
                rr = r["rowR"]                         # [128, n_ic*n_jg]
                rr = rr.reshape(128, n_ic, n_jg).min(axis=2)
                row = rr.T.reshape(-1)
                i0 = h * NI
                row_mins[b, i0:i0 + NI] = np.clip(
                    row + x2[b, i0:i0 + NI], 0.0, 100.0)
    out = (col_mins.mean(dtype=np.float64) + row_mins.mean(dtype=np.float64)) / B
    return np.asarray(out, dtype=F32)


_CACHE = {}
TRACE = False
LAST_RESULTS = None
SCHEME = "v2"
GW_RUN = GW


def kernel(corr_pred, corr_target):
    global LAST_RESULTS
    key = ("nc", SCHEME, GW_RUN)
    if key not in _CACHE:
        _CACHE[key] = build(scheme=SCHEME, gw=GW_RUN)
    nc = _CACHE[key]
    in_maps, x2, y2 = host_prep(corr_pred, corr_target, scheme=SCHEME, gw=GW_RUN)
    res = run_bass_kernel_spmd(nc, in_maps, core_ids=list(range(NCORES)),
                               trace=TRACE)
    LAST_RESULTS = res
    return combine(res.results, x2, y2, scheme=SCHEME, gw=GW_RUN)


# revision 15
# speedup vs baseline: 8.7311x; 8.7311x over previous
"""Correlation-cycle (Chamfer) loss kernel for Trainium2, 8 NeuronCores.

reference:  P[b,i,j] = ||x_i||^2 + ||y_j||^2 - 2 x_i.y_j   (x=corr_pred, y=corr_target)
            out = (mean_{b,j} min_i clip(P,0,100) + mean_{b,i} min_j clip(P,0,100)) / B

Sharding: B=4 batches x 2 i-halves -> 8 cores (no cross-core contention on
this stack, verified). Each core owns an x-half (2048 rows) and the full y
(4096 rows) of one batch.

Scheme "v4" (default, ~125us/exec vs 1.33ms baseline):
  PE:  per 512-col bank, TWO matmuls accumulate into PSUM:
         seed: ones[1,128]^T @ y2row[1,512]   (start=True,  stop=False)
         main: xT_chunk^T    @ (-2 yT)[...]   (start=False, stop=True)
       so psum = y2_j - 2 z_ij in fp32.
  ACT: one activation per [128 x 2048] group copies psum -> bf16 u with
       bias = x2_i (per-partition AP), i.e. u = P exactly.
  DVE: one tensor_tensor min per group accumulates colB = min_i P;
       per i-chunk, a 3D-AP tensor_reduce over the persistent u buffer
       yields rowm = min_j P (n_red=16 chunks overlap the tail).
Host: min over lanes/cores, clip(0,100) (clip commutes with min), means.

Numerics: nearly all mins clip to 100 for these inputs, so bf16 is far
inside the 2e-2 tolerance (measured rel err ~1.5e-07).

Engine budget per core (HW cost model): PE ~74us (256 MM), ACT ~61us
(32 copies), DVE ~70us (32 min + 16 reduces); exec ~125-129us.
Known-broken on this stack (do not use): native tensor_tensor_reduce
(crashes the core), any compute op on GpSimd/Pool except memset,
DMA writes to PSUM, matmul moving free dim > 512.

Older schemes kept for A/B: "hybrid" (the 1.33ms baseline), "v3"/"v3d".
"""

import numpy as np
import ml_dtypes

import concourse.bass as bass
import concourse.mybir as mybir
import concourse.tile as tile
from concourse import bacc
from concourse.bass_utils import run_bass_kernel_spmd

BF16 = ml_dtypes.bfloat16
F32 = np.float32

B, N, D = 4, 4096, 128
NCORES = 8
NI = N // 2          # per-core i range (half a batch)
NJ = N               # full j range
GW = 2048            # psum group width (4 banks)
MMW = 512            # matmul moving width (1 bank)
BIG = 1.0e38         # accumulator init (min identity; fits bf16)

AluOp = mybir.AluOpType
ActFn = mybir.ActivationFunctionType


def build(ni=NI, nj=NJ, gw=GW, reps=1, scheme="v3"):
    if scheme == "hybrid":
        return build_hybrid(ni, nj, min(gw, 2048), reps)
    if scheme in ("v3", "v3d"):
        return build_v3(ni, nj, min(gw, 2048), reps,
                        stt_pool=(scheme == "v3"))
    if scheme == "v4":
        return build_v4(ni, nj, min(gw, 2048), reps)
    n_ic = ni // 128
    n_jg = nj // gw
    nq = gw // MMW
    in_w = ni + 2 * nj                  # xT | m2yT | y2bc
    out_w = nj + n_ic * n_jg            # colB | rowB

    nc = bacc.Bacc("TRN2", target_bir_lowering=False, debug=False,
                   enable_asserts=False, num_devices=NCORES)
    f32 = mybir.dt.float32
    bf16 = mybir.dt.bfloat16

    in_d = nc.dram_tensor("inp", [128, in_w], bf16, kind="ExternalInput")
    x2c_d = nc.dram_tensor("x2c", [128, n_ic], f32, kind="ExternalInput")
    out_d = nc.dram_tensor("outp", [128, out_w], bf16, kind="ExternalOutput")

    psum_bufs = 2 if gw <= 2048 else 1

    with tile.TileContext(nc) as tc:
        with (
            tc.tile_pool(name="persist", bufs=1) as persist,
            tc.tile_pool(name="psum", bufs=psum_bufs, space="PSUM") as psum_pool,
            tc.tile_pool(name="u", bufs=2) as upool,
        ):
            inp = persist.tile([128, in_w], bf16, name="inp")
            x2c = persist.tile([128, n_ic], f32, name="x2c")
            outp = persist.tile([128, out_w], bf16, name="outp")
            xT = inp[:, 0:ni]
            m2yT = inp[:, ni:ni + nj]
            y2bc = inp[:, ni + nj:ni + 2 * nj]

            ck = 2048
            for c0 in range(0, in_w, ck):
                ce = min(c0 + ck, in_w)
                nc.sync.dma_start(out=inp[:, c0:ce], in_=in_d[:, c0:ce])
            nc.sync.dma_start(out=x2c[:, :], in_=x2c_d[:, :])

            def emit_body():
                for ic in range(n_ic):
                    for jg in range(n_jg):
                        sl = slice(jg * gw, (jg + 1) * gw)
                        psum = psum_pool.tile([128, gw], f32, tag="ps", name="ps")
                        for q in range(nq):
                            j0 = jg * gw + q * MMW
                            nc.tensor.matmul(
                                psum[:, q * MMW:(q + 1) * MMW],
                                xT[:, ic * 128:(ic + 1) * 128],
                                m2yT[:, j0:j0 + MMW])
                        u = upool.tile([128, gw], bf16, tag="u", name="u")
                        k = nj + ic * n_jg + jg
                        nc.vector.tensor_tensor_reduce(
                            u[:, :], psum[:, :], y2bc[:, sl], 1.0, BIG,
                            AluOp.add, AluOp.min, outp[:, k:k + 1])
                        if ic == 0:
                            nc.vector.tensor_scalar(
                                outp[:, sl], u[:, :], x2c[:, 0:1], None,
                                AluOp.add)
                        else:
                            nc.vector.scalar_tensor_tensor(
                                outp[:, sl], u[:, :], x2c[:, ic:ic + 1],
                                outp[:, sl], AluOp.add, AluOp.min)

            if reps > 1:
                with tc.For_i(0, reps, 1,
                              hint_engines=(mybir.EngineType.PE,
                                            mybir.EngineType.DVE)):
                    emit_body()
            else:
                emit_body()

            for c0 in range(0, out_w, ck):
                ce = min(c0 + ck, out_w)
                nc.sync.dma_start(out=out_d[:, c0:ce], in_=outp[:, c0:ce])

    nc.compile()
    return nc


def build_v3(ni, nj, gw, reps, stt_pool=True, n_red=4):
    """v3: TT(u=psum+y2bc) on DVE; col-min STT on GpSimd (stt_pool) or DVE;
    row mins via n_red bulk 3D tensor_reduce over the persistent u buffer."""
    n_ic = ni // 128
    n_jg = nj // gw
    nq = gw // MMW
    in_w = ni + 2 * nj                  # xT | m2yT | y2bc
    out_w = nj + n_ic                   # colB | rowm

    nc = bacc.Bacc("TRN2", target_bir_lowering=False, debug=False,
                   enable_asserts=False, num_devices=NCORES)
    f32 = mybir.dt.float32
    bf16 = mybir.dt.bfloat16

    in_d = nc.dram_tensor("inp", [128, in_w], bf16, kind="ExternalInput")
    x2c_d = nc.dram_tensor("x2c", [128, n_ic], f32, kind="ExternalInput")
    out_d = nc.dram_tensor("outp", [128, out_w], bf16, kind="ExternalOutput")

    stt = nc.gpsimd if stt_pool else nc.vector

    with tile.TileContext(nc) as tc:
        with (
            tc.tile_pool(name="persist", bufs=1) as persist,
            tc.tile_pool(name="psum", bufs=2, space="PSUM") as psum_pool,
        ):
            inp = persist.tile([128, in_w], bf16, name="inp")
            x2c = persist.tile([128, n_ic], f32, name="x2c")
            outp = persist.tile([128, out_w], bf16, name="outp")
            uall = persist.tile([128, n_ic * nj], bf16, name="uall")
            xT = inp[:, 0:ni]
            m2yT = inp[:, ni:ni + nj]
            y2bc = inp[:, ni + nj:ni + 2 * nj]

            ck = 2048
            for c0 in range(0, in_w, ck):
                ce = min(c0 + ck, in_w)
                nc.sync.dma_start(out=inp[:, c0:ce], in_=in_d[:, c0:ce])
            nc.sync.dma_start(out=x2c[:, :], in_=x2c_d[:, :])

            def emit_body():
                for ic in range(n_ic):
                    for jg in range(n_jg):
                        sl = slice(jg * gw, (jg + 1) * gw)
                        psum = psum_pool.tile([128, gw], f32, tag="ps", name="ps")
                        for q in range(nq):
                            j0 = jg * gw + q * MMW
                            nc.tensor.matmul(
                                psum[:, q * MMW:(q + 1) * MMW],
                                xT[:, ic * 128:(ic + 1) * 128],
                                m2yT[:, j0:j0 + MMW])
                        u = uall[:, ic * nj + jg * gw:ic * nj + (jg + 1) * gw]
                        nc.vector.tensor_tensor(
                            u, psum[:, :], y2bc[:, sl], AluOp.add)
                        if ic == 0:
                            stt.tensor_scalar(
                                outp[:, sl], u, x2c[:, 0:1], None, AluOp.add)
                        else:
                            stt.scalar_tensor_tensor(
                                outp[:, sl], u, x2c[:, ic:ic + 1],
                                outp[:, sl], AluOp.add, AluOp.min)
                    # after finishing a chunk of ics, bulk-reduce their rows
                    per = n_ic // n_red
                    if (ic + 1) % per == 0:
                        r0 = ic + 1 - per
                        view = uall[:, r0 * nj:(ic + 1) * nj].rearrange(
                            "p (i j) -> p i j", i=per, j=nj)
                        nc.vector.tensor_reduce(
                            outp[:, nj + r0:nj + ic + 1], view,
                            mybir.AxisListType.X, AluOp.min)

            if reps > 1:
                with tc.For_i(0, reps, 1,
                              hint_engines=(mybir.EngineType.PE,
                                            mybir.EngineType.DVE,
                                            mybir.EngineType.Pool)
                              if stt_pool else
                              (mybir.EngineType.PE, mybir.EngineType.DVE)):
                    emit_body()
            else:
                emit_body()

            half = out_w // 2
            nc.sync.dma_start(out=out_d[:, 0:half], in_=outp[:, 0:half])
            nc.sync.dma_start(out=out_d[:, half:out_w], in_=outp[:, half:out_w])

    nc.compile()
    return nc


def build_v4(ni, nj, gw, reps, n_red=16):
    """v4: psum = y2_j - 2 z  (y2 seeded by a K=1 ones x y2row matmul,
    start=True; main matmul accumulates with start=False).  ACT copies
    psum -> bf16 u with bias=x2_ic, so u = P directly.  DVE: plain
    tensor_tensor min for col accumulation + n_red bulk 3D row reduces.
    Engine balance: PE ~256 MM, ACT 32 copies, DVE 36 ops."""
    n_ic = ni // 128
    n_red = min(n_red, n_ic)
    n_jg = nj // gw
    nq = gw // MMW
    in_w = ni + nj                      # xT | m2yT
    out_w = nj + n_ic                   # colP | rowm   (both are min of P)

    nc = bacc.Bacc("TRN2", target_bir_lowering=False, debug=False,
                   enable_asserts=False, num_devices=NCORES)
    f32 = mybir.dt.float32
    bf16 = mybir.dt.bfloat16

    in_d = nc.dram_tensor("inp", [128, in_w], bf16, kind="ExternalInput")
    x2c_d = nc.dram_tensor("x2c", [128, n_ic], f32, kind="ExternalInput")
    y2r_d = nc.dram_tensor("y2r", [1, nj], bf16, kind="ExternalInput")
    out_d = nc.dram_tensor("outp", [128, out_w], bf16, kind="ExternalOutput")

    with tile.TileContext(nc) as tc:
        with (
            tc.tile_pool(name="persist", bufs=1) as persist,
            tc.tile_pool(name="psum", bufs=2, space="PSUM") as psum_pool,
        ):
            inp = persist.tile([128, in_w], bf16, name="inp")
            x2c = persist.tile([128, n_ic], f32, name="x2c")
            y2r = persist.tile([1, nj], bf16, name="y2r")
            ones = persist.tile([1, 128], bf16, name="ones")
            outp = persist.tile([128, out_w], bf16, name="outp")
            uall = persist.tile([128, n_ic * nj], bf16, name="uall")
            xT = inp[:, 0:ni]
            m2yT = inp[:, ni:ni + nj]

            ck = 2048
            for c0 in range(0, in_w, ck):
                ce = min(c0 + ck, in_w)
                nc.sync.dma_start(out=inp[:, c0:ce], in_=in_d[:, c0:ce])
            nc.sync.dma_start(out=x2c[:, :], in_=x2c_d[:, :])
            nc.sync.dma_start(out=y2r[:, :], in_=y2r_d[:, :])
            nc.vector.memset(ones[:, :], 1.0)

            def emit_body():
                for ic in range(n_ic):
                    for jg in range(n_jg):
                        sl = slice(jg * gw, (jg + 1) * gw)
                        psum = psum_pool.tile([128, gw], f32, tag="ps", name="ps")
                        for q in range(nq):
                            j0 = jg * gw + q * MMW
                            qs = slice(q * MMW, (q + 1) * MMW)
                            nc.tensor.matmul(
                                psum[:, qs], ones[:, :], y2r[:, j0:j0 + MMW],
                                start=True, stop=False, skip_group_check=True)
                            nc.tensor.matmul(
                                psum[:, qs],
                                xT[:, ic * 128:(ic + 1) * 128],
                                m2yT[:, j0:j0 + MMW],
                                start=False, stop=True, skip_group_check=True)
                        u = uall[:, ic * nj + jg * gw:ic * nj + (jg + 1) * gw]
                        nc.scalar.activation(u, psum[:, :], ActFn.Identity,
                                             bias=x2c[:, ic:ic + 1], scale=1.0)
                        if ic == 0:
                            nc.vector.tensor_copy(outp[:, sl], u)
                        else:
                            nc.vector.tensor_tensor(
                                outp[:, sl], u, outp[:, sl], AluOp.min)
                    per = n_ic // n_red
                    if (ic + 1) % per == 0:
                        r0 = ic + 1 - per
                        view = uall[:, r0 * nj:(ic + 1) * nj].rearrange(
                            "p (i j) -> p i j", i=per, j=nj)
                        nc.vector.tensor_reduce(
                            outp[:, nj + r0:nj + ic + 1], view,
                            mybir.AxisListType.X, AluOp.min)

            if reps > 1:
                with tc.For_i(0, reps, 1,
                              hint_engines=(mybir.EngineType.PE,
                                            mybir.EngineType.DVE,
                                            mybir.EngineType.Activation)):
                    emit_body()
            else:
                emit_body()

            half = out_w // 2
            nc.sync.dma_start(out=out_d[:, 0:half], in_=outp[:, 0:half])
            nc.sync.dma_start(out=out_d[:, half:out_w], in_=outp[:, half:out_w])

    nc.compile()
    return nc


def build_hybrid(ni, nj, gw, reps):
    """Previous baseline: 3 DVE ops/group, separate dram tensors."""
    n_ic = ni // 128
    n_jg = nj // gw

    nc = bacc.Bacc("TRN2", target_bir_lowering=False, debug=False,
                   enable_asserts=False, num_devices=NCORES)
    f32 = mybir.dt.float32
    bf16 = mybir.dt.bfloat16

    xT_d = nc.dram_tensor("xT", [128, ni], bf16, kind="ExternalInput")
    m2yT_d = nc.dram_tensor("m2yT", [128, nj], bf16, kind="ExternalInput")
    x2c_d = nc.dram_tensor("x2c", [128, n_ic], f32, kind="ExternalInput")
    y2bc_d = nc.dram_tensor("y2bc", [128, nj], bf16, kind="ExternalInput")
    colB_d = nc.dram_tensor("colB", [128, nj], bf16, kind="ExternalOutput")
    rowR_d = nc.dram_tensor("rowR", [128, n_ic * n_jg], f32, kind="ExternalOutput")

    with tile.TileContext(nc) as tc:
        with (
            tc.tile_pool(name="persist", bufs=1) as persist,
            tc.tile_pool(name="psum", bufs=2, space="PSUM") as psum_pool,
            tc.tile_pool(name="u", bufs=3) as upool,
        ):
            xT = persist.tile([128, ni], bf16, name="xT")
            m2yT = persist.tile([128, nj], bf16, name="m2yT")
            x2c = persist.tile([128, n_ic], f32, name="x2c")
            y2bc = persist.tile([128, nj], bf16, name="y2bc")
            colB = persist.tile([128, nj], bf16, name="colB")
            rowR = persist.tile([128, n_ic * n_jg], f32, name="rowR")

            nc.sync.dma_start(out=xT[:, :], in_=xT_d[:, :])
            ck = min(2048, nj)
            for c0 in range(0, nj, ck):
                nc.sync.dma_start(out=m2yT[:, c0:c0 + ck], in_=m2yT_d[:, c0:c0 + ck])
                nc.sync.dma_start(out=y2bc[:, c0:c0 + ck], in_=y2bc_d[:, c0:c0 + ck])
            nc.sync.dma_start(out=x2c[:, :], in_=x2c_d[:, :])
            nc.vector.memset(colB[:, :], BIG)

            def emit_body():
                for ic in range(n_ic):
                    for jg in range(n_jg):
                        sl = slice(jg * gw, (jg + 1) * gw)
                        psum = psum_pool.tile([128, gw], f32, tag="ps", name="ps")
                        for q in range(gw // MMW):
                            j0 = jg * gw + q * MMW
                            nc.tensor.matmul(
                                psum[:, q * MMW:(q + 1) * MMW],
                                xT[:, ic * 128:(ic + 1) * 128],
                                m2yT[:, j0:j0 + MMW])
                        u = upool.tile([128, gw], bf16, tag="u", name="u")
                        nc.vector.tensor_tensor(
                            u[:, :], psum[:, :], y2bc[:, sl], AluOp.add)
                        k = ic * n_jg + jg
                        nc.vector.tensor_reduce(
                            rowR[:, k:k + 1], u[:, :],
                            mybir.AxisListType.X, AluOp.min)
                        nc.vector.scalar_tensor_tensor(
                            colB[:, sl], u[:, :], x2c[:, ic:ic + 1],
                            colB[:, sl], AluOp.add, AluOp.min)

            if reps > 1:
                with tc.For_i(0, reps, 1,
                              hint_engines=(mybir.EngineType.PE,
                                            mybir.EngineType.DVE)):
                    emit_body()
            else:
                emit_body()

            for c0 in range(0, nj, ck):
                nc.sync.dma_start(out=colB_d[:, c0:c0 + ck], in_=colB[:, c0:c0 + ck])
            nc.sync.dma_start(out=rowR_d[:, :], in_=rowR[:, :])

    nc.compile()
    return nc


def host_prep(x, y, scheme="v2", gw=GW):
    """Per-core input maps. Core c: batch c//2, i-half c%2."""
    x = np.ascontiguousarray(np.asarray(x, F32))
    y = np.ascontiguousarray(np.asarray(y, F32))
    x16 = x.astype(BF16)
    y16 = y.astype(BF16)
    m2y16 = (y16.astype(F32) * -2.0).astype(BF16)          # exact in bf16
    x2 = (x16.astype(F32) ** 2).sum(-1)                    # [B, N]
    y2 = (y16.astype(F32) ** 2).sum(-1)
    n_ic = NI // 128
    in_maps = []
    for c in range(NCORES):
        b, h = divmod(c, 2)
        i0 = h * NI
        if scheme == "v4":
            xTc = x16[b, i0:i0 + NI, :].T                          # [128, NI]
            inp = np.concatenate([xTc, m2y16[b].T], axis=1)
            m = {"inp": np.ascontiguousarray(inp),
                 "x2c": np.ascontiguousarray(
                     x2[b, i0:i0 + NI].reshape(n_ic, 128).T),
                 "y2r": np.ascontiguousarray(
                     y2[b].astype(BF16).reshape(1, N))}
        elif scheme in ("v2", "v3", "v3d"):
            xTc = x16[b, i0:i0 + NI, :].T                          # [128, NI]
            y2bc = np.broadcast_to(y2[b].astype(BF16), (128, N))   # [128, N]
            inp = np.concatenate([xTc, m2y16[b].T, y2bc], axis=1)
            m = {"inp": np.ascontiguousarray(inp),
                 "x2c": np.ascontiguousarray(
                     x2[b, i0:i0 + NI].reshape(n_ic, 128).T)}
        else:
            m = {
                "xT": np.ascontiguousarray(x16[b, i0:i0 + NI, :].T),
                "m2yT": np.ascontiguousarray(m2y16[b].T),
                "x2c": np.ascontiguousarray(
                    x2[b, i0:i0 + NI].reshape(n_ic, 128).T),
                "y2bc": np.ascontiguousarray(
                    np.broadcast_to(y2[b].astype(BF16), (128, N))),
            }
        in_maps.append(m)
    return in_maps, x2, y2


def combine(results, x2, y2, scheme="v2", gw=GW):
    n_ic = NI // 128
    n_jg = N // gw
    col_mins = np.empty((B, N), F32)
    row_mins = np.empty((B, N), F32)
    for b in range(B):
        cores = [results[2 * b], results[2 * b + 1]]
        if scheme in ("v2", "v3", "v3d", "v4"):
            col = np.minimum.reduce(
                [r["outp"][:, :N].astype(F32).min(0) for r in cores])
            col_mins[b] = np.clip(col, 0.0, 100.0)
            for h, r in enumerate(cores):
                rr = r["outp"][:, N:].astype(F32)      # [128, n_ic(*n_jg)]
                if scheme == "v2":
                    rr = rr.reshape(128, n_ic, n_jg).min(axis=2)
                row = rr.T.reshape(-1)                 # [NI], i = ic*128 + lane
                i0 = h * NI
                if scheme == "v4":                     # rowm is min_j P already
                    row_mins[b, i0:i0 + NI] = np.clip(row, 0.0, 100.0)
                else:
                    row_mins[b, i0:i0 + NI] = np.clip(
                        row + x2[b, i0:i0 + NI], 0.0, 100.0)
        else:
            col = np.minimum.reduce([r["colB"].astype(F32).min(0) for r in cores])
            col_mins[b] = np.clip(col, 0.0, 100.0)
            for h, r in enumerate(cores):
                rr = r["rowR"]                         # [128, n_ic*n_jg]
                rr = rr.reshape(128, n_ic, n_jg).min(axis=2)
                row = rr.T.reshape(-1)
                i0 = h * NI
                row_mins[b, i0:i0 + NI] = np.clip(
                    row + x2[b, i0:i0 + NI], 0.0, 100.0)
    out = (col_mins.mean(dtype=np.float64) + row_mins.mean(dtype=np.float64)) / B
    return np.asarray(out, dtype=F32)


_CACHE = {}
TRACE = False
LAST_RESULTS = None
SCHEME = "v4"
GW_RUN = GW


def kernel(corr_pred, corr_target):
    global LAST_RESULTS
    key = ("nc", SCHEME, GW_RUN)
    if key not in _CACHE:
        _CACHE[key] = build(scheme=SCHEME, gw=GW_RUN)
    nc = _CACHE[key]
    in_maps, x2, y2 = host_prep(corr_pred, corr_target, scheme=SCHEME, gw=GW_RUN)
    res = run_bass_kernel_spmd(nc, in_maps, core_ids=list(range(NCORES)),
                               trace=TRACE)
    LAST_RESULTS = res
    return combine(res.results, x2, y2, scheme=SCHEME, gw=GW_RUN)
